# revision 16
# baseline (speedup 1.0000x reference)
"""Bass/Tile TRN2 kernel for nn_DecoderGroupedQueryHeadAttentionAlibi.

Sharding (8 cores): core = (b, g) with b = core//2 in [0,4) (batch),
g = core%2 (head parity). Slot i in [0,8) on group g computes global head
2*i + g; kv head of slot i is g + 2*(i%2). The host sums the two parity
partials of the row-sliced output projection and adds bproj.

Per-core device program (scoresT layout: [s_partitions, t_free]):
  - q/k/v projections from host-pretransposed xT/weight tiles (bf16),
    woven between attention iterations so the scalar engine never idles
  - scores as 64x64 PE-array quads (4 concurrent tile_position matmuls)
  - per (slot, s-tile): ACT exp with the alibi linear bias folded into the
    per-partition activation bias; fully-future tiles use bias=0 (the alibi
    bias is zero above the diagonal), others get a DVE fix on the future
    prefix plus Toeplitz multipliers on diag/past regions
  - attn@v accumulation in psum [65, 2048]; row 64 (ones column in v) is
    the softmax denominator
  - per head-pair: denominator reciprocal, normalize, and a two-pass output
    projection (pass A for early pairs hidden under later heads' attention).
"""

import math
import numpy as np

# ---- problem constants (hardcoded; kernel.py must be self-contained) ----
B, T, C = 4, 2048, 1024
N_HEAD, N_KV_HEAD, HEAD_DIM = 16, 4, 64
NH = 8            # head slots per core
ST = T // 128     # 16 s-tiles
NCH = T // 512    # 4 t-chunks
KCT = C // 128    # 8 contraction tiles of 128
CUT_MARGIN = 10.0  # exp(-10) ~ 4.5e-5: dropped mass is ~1e-4 of denom

_START = 2.0 ** (-2.0 ** (-(math.log2(N_HEAD) - 3.0)))  # 0.7071...

SLOT_ORDER = [4, 5, 0, 1, 6, 7, 3, 2]


def _head_of_slot(i: int, g: int) -> int:
    return 2 * i + g


def _a_of_head(h: int) -> float:
    return (_START ** (h + 1)) / math.sqrt(HEAD_DIM)


# Loop bounds must be identical on every core (SPMD): use the widest cutoff
# over g for each head slot (g=1 heads have smaller slopes -> wider bands).
_CUTOFF = [CUT_MARGIN / min(_a_of_head(_head_of_slot(i, 0)),
                            _a_of_head(_head_of_slot(i, 1)))
           for i in range(NH)]
_N_EFF = [[min(NCH, int((128 * j + _CUTOFF[i]) // 512) + 1)
           for j in range(ST)] for i in range(NH)]
_J_FIRST = [[min(j for j in range(ST) if _N_EFF[i][j] > tcn)
             for tcn in range(NCH)] for i in range(NH)]
# per-slot Toeplitz table widths (max index read is cutoff+512, cap 2048)
_WREP_W = [min(2048, int(math.ceil(_CUTOFF[i])) + 512) for i in range(NH)]
_WREP_OFF = [sum(_WREP_W[:i]) for i in range(NH)]

_NC_CACHE = {}


def _split_multiwait(nc, mybir, max_waits=1):
    """walrus in this env encodes at most one sync-wait per instruction;
    split extras onto same-engine NoOps emitted just before."""
    for f in nc.m.functions:
        for bb in f.blocks:
            new = []
            for ins in bb.instructions:
                si = ins.sync_info
                conds = list(si.on_wait) if si is not None else []
                if len(conds) > max_waits:
                    for cond in conds[:-max_waits]:
                        n = mybir.InstNoOp(
                            name=nc.get_next_instruction_name(), ins=[], outs=[])
                        n.engine = ins.engine
                        n.sync_info = mybir.SyncInfo(on_wait=[cond], on_update=[])
                        new.append(n)
                    si.on_wait = conds[-max_waits:]
                new.append(ins)
            bb.instructions = new


def _build_nc():
    if "nc" in _NC_CACHE:
        return _NC_CACHE["nc"]
    from contextlib import ExitStack
    import concourse.bass as bass
    import concourse.tile as tile
    from concourse import mybir

    f32 = mybir.dt.float32
    bf16 = mybir.dt.bfloat16
    AF = mybir.ActivationFunctionType
    MUL = mybir.AluOpType.mult
    ADD = mybir.AluOpType.add
    MIN = mybir.AluOpType.min

    nc = bass.Bass()

    xT_d = nc.dram_tensor("xT", [C, T], bf16, kind="ExternalInput")
    wq_d = nc.dram_tensor("wqT", [C, NH * 64], bf16, kind="ExternalInput")
    wk_d = nc.dram_tensor("wkT", [C, 128], bf16, kind="ExternalInput")
    wv_d = nc.dram_tensor("wvT", [C, 128], bf16, kind="ExternalInput")
    wp_d = nc.dram_tensor("wpT", [NH * 64, C], bf16, kind="ExternalInput")
    wrep_d = nc.dram_tensor("wrep", [NH, 128, 2048], bf16, kind="ExternalInput")
    u_d = nc.dram_tensor("usb", [128, NH], f32, kind="ExternalInput")
    bias_d = nc.dram_tensor("biassb", [128, NH], f32, kind="ExternalInput")
    out_d = nc.dram_tensor("out", [T, C], f32, kind="ExternalOutput")

    xT_r = xT_d.rearrange("(k p) t -> p k t", p=128)
    wq_r = wq_d.rearrange("(k p) e -> p k e", p=128)
    wrep_r = wrep_d.rearrange("h p w -> p h w")

    with ExitStack() as es:
        tc = es.enter_context(tile.TileContext(nc))
        const = es.enter_context(tc.tile_pool(name="const", bufs=1))
        work = es.enter_context(tc.tile_pool(name="work", bufs=2))
        ebufp = es.enter_context(tc.tile_pool(name="ebuf", bufs=3))
        dstgp = es.enter_context(tc.tile_pool(name="dstg", bufs=1))
        outp = es.enter_context(tc.tile_pool(name="outp", bufs=2))
        dramd = es.enter_context(tc.tile_pool(name="dramd", bufs=1, space="DRAM"))
        ps = es.enter_context(tc.tile_pool(name="ps", bufs=1, space="PSUM"))
        ph1 = es.enter_context(tc.tile_pool(name="ph1", bufs=1))

        # ---- persistent tiles ----
        wp = const.tile([128, 4, C], bf16)
        wrep = const.tile([128, sum(_WREP_W)], bf16)
        usb = const.tile([128, NH], f32)
        biassb = const.tile([128, NH], f32)
        kRep = const.tile([128, 2, T], bf16)     # kv on both halves
        v_sb = const.tile([128, ST, 130], bf16)  # [s, j, (v_kv0|1|v_kv1|1)]
        qRep = const.tile([128, NH, T], bf16)    # slot i on both halves
        outT = const.tile([128, 4, T], bf16)     # [(2 slots d), pair, t]
        osbA = const.tile([128, ST, 1024], bf16)  # pass-A outproj partial
        warm = const.tile([128, 1], f32)

        ddrow = dramd.tile([NH, T], bf16)
        rdram = dramd.tile([NH, T], bf16)
        rd3 = rdram.rearrange("i (a b) -> i a b", b=128)

        # transient projection inputs
        xT = ph1.tile([128, KCT, T], bf16)
        wk = ph1.tile([128, KCT, 128], bf16)
        wq = ph1.tile([128, KCT, NH * 64], bf16)
        wv = ph1.tile([128, KCT, 128], bf16)

        def wr(i):  # per-slot Toeplitz slice accessor
            return wrep[:, _WREP_OFF[i]:_WREP_OFF[i] + _WREP_W[i]]

        # ---- prologue DMAs (dependency-critical order) ----
        # xT t-block-major so the first projections start after ~1MB, not 4MB
        nc.sync.dma_start(out=wk, in_=wk_d.rearrange("(k p) e -> p k e", p=128))
        # force the exp table load early so it overlaps the prologue DMAs
        nc.vector.memset(warm, 0.0)
        nc.scalar.activation(warm, warm, AF.Exp, bias=0.0, scale=1.0)
        for kc in range(KCT):
            nc.sync.dma_start(out=xT[:, kc, 0:512], in_=xT_r[:, kc, 0:512])
        nc.gpsimd.dma_start(out=wq[:, :, 256:384], in_=wq_r[:, :, 256:384])
        nc.gpsimd.dma_start(out=wv, in_=wv_d.rearrange("(k p) e -> p k e", p=128))
        for tb in range(1, 4):
            for kc in range(KCT):
                nc.sync.dma_start(out=xT[:, kc, 512 * tb:512 * (tb + 1)],
                                  in_=xT_r[:, kc, 512 * tb:512 * (tb + 1)])
        nc.gpsimd.dma_start(out=usb, in_=u_d[:])
        nc.gpsimd.dma_start(out=biassb, in_=bias_d[:])
        nc.gpsimd.dma_start(out=wq[:, :, 0:128], in_=wq_r[:, :, 0:128])
        for pos in range(NH):
            i = SLOT_ORDER[pos]
            nc.gpsimd.dma_start(out=wr(i), in_=wrep_r[:, i, 0:_WREP_W[i]])
        nc.gpsimd.dma_start(out=wq[:, :, 384:512], in_=wq_r[:, :, 384:512])
        nc.gpsimd.dma_start(out=wq[:, :, 128:256], in_=wq_r[:, :, 128:256])
        nc.gpsimd.dma_start(out=wp, in_=wp_d.rearrange("(k p) e -> p k e", p=128))
        nc.vector.memset(v_sb[:, :, 64], 1.0)
        nc.vector.memset(v_sb[:, :, 129], 1.0)

        # ---- projection work items ----
        def k_proj_chunk(sc):
            pk = ps.tile([128, 1024], f32, tag="S", bufs=2)
            for kc in range(KCT):
                nc.tensor.matmul(
                    pk[:, 0:512], lhsT=wk[:, kc, :],
                    rhs=xT[:, kc, 512 * sc:512 * (sc + 1)],
                    start=(kc == 0), stop=(kc == KCT - 1))
            sl = slice(512 * sc, 512 * (sc + 1))
            nc.vector.tensor_copy(kRep[0:64, 0, sl], pk[0:64, 0:512])
            nc.vector.tensor_copy(kRep[64:128, 1, sl], pk[64:128, 0:512])
            nc.sync.dma_start(out=kRep[64:128, 0, sl], in_=kRep[0:64, 0, sl])
            nc.sync.dma_start(out=kRep[0:64, 1, sl], in_=kRep[64:128, 1, sl])

        def q_proj_chunk(p, tcn):
            pq = ps.tile([128, 1024], f32, tag="S", bufs=2)
            for kc in range(KCT):
                nc.tensor.matmul(
                    pq[:, 0:512], lhsT=wq[:, kc, 128 * p:128 * (p + 1)],
                    rhs=xT[:, kc, 512 * tcn:512 * (tcn + 1)],
                    start=(kc == 0), stop=(kc == KCT - 1))
            sl = slice(512 * tcn, 512 * (tcn + 1))
            nc.vector.tensor_copy(qRep[0:64, 2 * p, sl], pq[0:64, 0:512])
            nc.vector.tensor_copy(qRep[64:128, 2 * p + 1, sl], pq[64:128, 0:512])
            nc.sync.dma_start(out=qRep[64:128, 2 * p, sl],
                              in_=qRep[0:64, 2 * p, sl])
            nc.sync.dma_start(out=qRep[0:64, 2 * p + 1, sl],
                              in_=qRep[64:128, 2 * p + 1, sl])

        def v_proj_tile(st):
            pv = ps.tile([128, 1024], f32, tag="S", bufs=2)
            for kc in range(KCT):
                nc.tensor.matmul(
                    pv[:, 0:128], lhsT=xT[:, kc, 128 * st:128 * (st + 1)],
                    rhs=wv[:, kc, :],
                    start=(kc == 0), stop=(kc == KCT - 1))
            nc.vector.tensor_copy(v_sb[:, st, 0:64], pv[:, 0:64])
            nc.vector.tensor_copy(v_sb[:, st, 65:129], pv[:, 64:128])

        def passA_tile(tt):
            # output projection over the three early-finishing pairs
            pp = ps.tile([128, 1024], f32, tag="S", bufs=2)
            for ec in range(2):
                for n_, kt in enumerate([2, 0, 3]):
                    nc.tensor.matmul(
                        pp[:, 512 * ec:512 * (ec + 1)],
                        lhsT=outT[:, kt, 128 * tt:128 * (tt + 1)],
                        rhs=wp[:, kt, 512 * ec:512 * (ec + 1)],
                        start=(n_ == 0), stop=(n_ == 2),
                        skip_group_check=True)
            nc.vector.tensor_copy(osbA[:, tt, :], pp)

        # work item lists per slot position: (closure, est_tensor_ns).
        # Items are emitted at iteration starts, scheduled proportionally to
        # accumulated chunk mass, with a lead so every item precedes the
        # first attention emission that reads its output.
        QN, KN, VN, PN = 1750, 1750, 800, 900
        slot_items = [[] for _ in range(NH)]
        # pos 0 = slot 4: v tiles 1..15 and k chunks 1..3 forced here
        slot_items[0] = ([(lambda st=st: v_proj_tile(st), VN) for st in range(1, 4)]
                         + [(lambda: k_proj_chunk(1), KN)]
                         + [(lambda st=st: v_proj_tile(st), VN) for st in range(4, 7)]
                         + [(lambda: k_proj_chunk(2), KN)]
                         + [(lambda st=st: v_proj_tile(st), VN) for st in range(7, 10)]
                         + [(lambda: k_proj_chunk(3), KN)]
                         + [(lambda st=st: v_proj_tile(st), VN) for st in range(10, 16)])
        # pos 1 = slot 5: q pair 0 (needed by slots 0,1 at pos 2,3)
        slot_items[1] = [(lambda t=t: q_proj_chunk(0, t), QN) for t in range(NCH)]
        # pos 2,3 = slots 0,1: q pair 3 (for slots 6,7)
        slot_items[2] = [(lambda t=t: q_proj_chunk(3, t), QN) for t in range(2)]
        slot_items[3] = [(lambda t=t: q_proj_chunk(3, t), QN) for t in range(2, 4)]
        # pos 4,5 = slots 6,7: q pair 1 (for slots 3,2)
        slot_items[4] = [(lambda t=t: q_proj_chunk(1, t), QN) for t in range(2)]
        slot_items[5] = [(lambda t=t: q_proj_chunk(1, t), QN) for t in range(2, 4)]
        # pos 6,7 = slots 3,2: output projection pass A (pairs 2, 0, 3)
        slot_items[6] = [(lambda tt=tt: passA_tile(tt), PN) for tt in range(8)]
        slot_items[7] = [(lambda tt=tt: passA_tile(tt), PN) for tt in range(8, 16)]

        def finish_slot(i):
            # slot i's denom row is in ddrow: recip + normalize outT half
            p, hh = i // 2, i % 2
            dstb = dstgp.tile([16, 128], bf16, tag="dstb", bufs=2)
            nc.gpsimd.dma_start(out=dstb,
                                in_=ddrow[i].rearrange("(a b) -> a b", b=128))
            dstf = dstgp.tile([16, 128], f32, tag="dstf", bufs=2)
            nc.vector.tensor_copy(dstf, dstb)
            rstf = dstgp.tile([16, 128], f32, tag="rstf", bufs=2)
            nc.vector.reciprocal(rstf, dstf)
            rstb = dstgp.tile([16, 128], bf16, tag="rstb", bufs=2)
            nc.vector.tensor_copy(rstb, rstf)
            nc.gpsimd.dma_start(out=rd3[i], in_=rstb[0:16, :])
            rrep = work.tile([128, T], bf16, tag="rrep")
            src = rdram[i:i + 1, :]
            src = bass.AP(tensor=src.tensor, offset=src.offset,
                          ap=[[0, 64]] + list(src.ap)[1:])
            nc.gpsimd.dma_start(out=rrep[64 * hh:64 * hh + 64, :], in_=src)
            nc.vector.tensor_tensor(outT[64 * hh:64 * hh + 64, p, :],
                                    outT[64 * hh:64 * hh + 64, p, :],
                                    rrep[64 * hh:64 * hh + 64, :], MUL)

        # ---- prologue compute: k(0), q pair2 (slot 4 needs all 4), v(0) ----
        k_proj_chunk(0)
        for t in range(NCH):
            q_proj_chunk(2, t)
        v_proj_tile(0)

        # ---- attention slots with woven work items ----
        for pos in range(NH):
            i = SLOT_ORDER[pos]
            p, half = i // 2, i % 2
            items = list(slot_items[pos])
            n_items = len(items)
            emitted = 0
            cum = []
            tot = 0
            for j in range(ST):
                tot += _N_EFF[i][j]
                cum.append(tot)

            pa = ps.tile([128, T], f32, tag="pa", bufs=1)
            dmin = work.tile([128, 128], bf16, tag="dmin")
            nc.vector.tensor_scalar(dmin, wr(i)[:, 0:128],
                                    usb[:, i:i + 1], None, MIN)
            for j in range(ST):
                # weave: emit work items proportionally to chunk mass with a
                # one-iteration lead (an item must precede the first
                # attention emission that depends on it)
                want = min(n_items, n_items * cum[min(j + 1, ST - 1)] // tot)
                woven_ns = 0
                while emitted < want:
                    fn, est = items[emitted]
                    fn()
                    woven_ns += est
                    emitted += 1
                ne = _N_EFF[i][j]
                lo = 128 * j          # t < lo : future region (bias 0)
                hi = 128 * (j + 1)    # t >= hi: past region (Toeplitz)
                # HAM filler: keep PE duty high enough to hold K=8/8; dummy
                # matmuls land in the S tile region the quads then overwrite
                ntiles = 1 if ne <= 2 else 2
                act_ns = (512 * ne + 210 * ntiles) / 1.2
                ten_ns = ne * 340 + 160
                n_dummy = min(4, max(0, int(
                    (0.88 * act_ns - ten_ns - woven_ns) // 110)))
                n_dummy = 0  # disabled: suspected psum-group hazard
                # dummies write unused pa partitions 96:128 — no data deps,
                # so they fill PE idle immediately instead of queueing behind
                # the S-ring wait
                for _ in range(n_dummy):
                    nc.tensor.matmul(
                        pa[96:128, 0:256],
                        lhsT=kRep[0:64, half, 0:32],
                        rhs=qRep[0:64, i, 0:256],
                        start=True, stop=True,
                        tile_position=(0, 96),
                        skip_group_check=True)
                E = ebufp.tile([128, T], bf16, tag="E")
                for sh in range(2):
                    c0, c1 = 2 * sh, min(ne, 2 * sh + 2)
                    if c0 >= c1:
                        continue
                    base, top = 512 * c0, 512 * c1
                    S = ps.tile([128, 1024], f32, tag="S", bufs=2)
                    for tcn in range(c0, c1):
                        rh = 64 * (tcn - c0)
                        o = 512 * (tcn - c0)
                        for kh in range(2):
                            nc.tensor.matmul(
                                S[64 * kh:64 * kh + 64, o:o + 512],
                                lhsT=kRep[rh:rh + 64, half,
                                          128 * j + 64 * kh:
                                          128 * j + 64 * kh + 64],
                                rhs=qRep[rh:rh + 64, i,
                                         512 * tcn:512 * (tcn + 1)],
                                start=True, stop=True)
                    wv_ = top - base
                    if lo >= top:
                        # whole tile is future: alibi bias is 0 there
                        nc.scalar.activation(
                            E[:, base:top], S[:, 0:wv_], AF.Exp,
                            bias=0.0, scale=0.125)
                    else:
                        nc.scalar.activation(
                            E[:, base:top], S[:, 0:wv_], AF.Exp,
                            bias=biassb[:, i:i + 1], scale=0.125)
                        if lo > base:  # future prefix: cancel the bias
                            nc.vector.tensor_scalar(
                                E[:, base:lo], E[:, base:lo],
                                usb[:, i:i + 1], None, MUL)
                        if lo >= base:  # diagonal tile lives here
                            nc.vector.tensor_tensor(
                                E[:, lo:hi], E[:, lo:hi], dmin, MUL)
                        seg0 = max(hi, base)
                        if seg0 < top:  # past region: Toeplitz multiplier
                            nc.vector.tensor_tensor(
                                E[:, seg0:top], E[:, seg0:top],
                                wr(i)[:, 128 + seg0 - hi:128 + top - hi],
                                MUL)
                    for tcn in range(c0, c1):
                        nc.tensor.matmul(
                            pa[0:65, 512 * tcn:512 * (tcn + 1)],
                            lhsT=v_sb[:, j, 65 * half:65 * half + 65],
                            rhs=E[:, 512 * tcn:512 * (tcn + 1)],
                            start=(j == _J_FIRST[i][tcn]), stop=(j == ST - 1),
                            skip_group_check=True)
            while emitted < n_items:
                items[emitted][0]()
                emitted += 1

            # copy-out: rows 0:64 -> outT half; row 64 -> denom
            st65 = dstgp.tile([65, T], bf16, tag="st65")
            nc.vector.tensor_copy(st65, pa[0:65, :])
            nc.sync.dma_start(out=outT[64 * half:64 * half + 64, p, :],
                              in_=st65[0:64, :])
            nc.sync.dma_start(out=ddrow[i:i + 1, :], in_=st65[64:65, :])
            finish_slot(i)

        # ---- output projection pass B (pair 1) + final add + store ----
        for tt in range(ST):
            pp = ps.tile([128, 1024], f32, tag="S", bufs=2)
            for ec in range(2):
                nc.tensor.matmul(
                    pp[:, 512 * ec:512 * (ec + 1)],
                    lhsT=outT[:, 1, 128 * tt:128 * (tt + 1)],
                    rhs=wp[:, 1, 512 * ec:512 * (ec + 1)],
                    start=True, stop=True)
            osb = outp.tile([128, C], f32, tag="osb")
            nc.vector.tensor_tensor(osb, pp, osbA[:, tt, :], ADD)
            eng = nc.sync if tt % 2 == 0 else nc.gpsimd
            eng.dma_start(out=out_d[128 * tt:128 * (tt + 1), :], in_=osb)

    _split_multiwait(nc, mybir)
    _NC_CACHE["nc"] = nc
    return nc


def _prep_core_inputs(x, Wq, Wkv, Wproj, b, g):
    import ml_dtypes
    bf = ml_dtypes.bfloat16
    heads = [_head_of_slot(i, g) for i in range(NH)]
    xT = np.ascontiguousarray(x[b].T).astype(bf)                      # [C, T]
    wq_cols = np.concatenate([Wq[64 * h:64 * (h + 1)] for h in heads], axis=0)
    wqT = np.ascontiguousarray(wq_cols.T).astype(bf)                  # [C, 512]
    kv_rows = np.concatenate([np.arange(64 * kv, 64 * (kv + 1))
                              for kv in (g, g + 2)])
    wkT = np.ascontiguousarray(Wkv[kv_rows].T).astype(bf)             # [C, 128]
    wvT = np.ascontiguousarray(Wkv[256 + kv_rows].T).astype(bf)
    cols = np.concatenate([np.arange(64 * h, 64 * (h + 1)) for h in heads])
    wpT = np.ascontiguousarray(Wproj[:, cols].T).astype(bf)           # [512, C]

    s_in = np.arange(128, dtype=np.float64)
    wrep = np.zeros((NH, 128, 2048), dtype=bf)
    u = np.empty((128, NH), dtype=np.float32)
    bias = np.empty((128, NH), dtype=np.float32)
    idx = np.arange(2048, dtype=np.float64)
    for i, h in enumerate(heads):
        a = _a_of_head(h)
        w = _WREP_W[i]
        wrep[i, :, :w] = np.exp(-a * (idx[:w] - 127.0))[None, :].astype(np.float32)
        u[:, i] = np.exp(a * (127.0 - s_in)).astype(np.float32)
        bias[:, i] = (a * (s_in - 127.0)).astype(np.float32)
    return {"xT": xT, "wqT": wqT, "wkT": wkT, "wvT": wvT, "wpT": wpT,
            "wrep": wrep, "usb": u, "biassb": bias}


def kernel(x, Wq, Wkv, Wproj, bproj):
    from concourse.bass_utils import run_bass_kernel_spmd
    x = np.asarray(x, dtype=np.float32)
    Wq = np.asarray(Wq, dtype=np.float32)
    Wkv = np.asarray(Wkv, dtype=np.float32)
    Wproj = np.asarray(Wproj, dtype=np.float32)
    bproj = np.asarray(bproj, dtype=np.float32)

    nc = _build_nc()
    in_maps = [_prep_core_inputs(x, Wq, Wkv, Wproj, c // 2, c % 2)
               for c in range(8)]
    res = run_bass_kernel_spmd(nc, in_maps, core_ids=list(range(8)))
    out = np.zeros((B, T, C), dtype=np.float32)
    for c in range(8):
        out[c // 2] += res.results[c]["out"]
    out += bproj[None, None, :]
    return out


# revision 19
# speedup vs baseline: 1.3135x; 1.3135x over previous
"""Bass/Tile TRN2 kernel for nn_DecoderGroupedQueryHeadAttentionAlibi.

Sharding (8 cores): core = (b, g) with b = core//2 in [0,4) (batch),
g = core%2 (head-half). Each core computes 8 of 16 query heads (those with
h%4 in {2g, 2g+1}) for its batch, plus the corresponding row-slice of the
output projection; the host sums the two half partials and adds bproj.

Per-core device program (layout A, scoresT = [s_partitions, t_free]):
  - projections q/k/v from host-pretransposed xT/weight tiles (bf16 matmuls)
  - per (head, s-tile): scoresT psum [128,2048] -> ACT exp (alibi linear bias
    folded into the activation per-partition bias) -> DVE multiplier using a
    Toeplitz exp table sliced at a per-s-tile offset -> attn@v accumulation
    in psum [65,2048] where row 64 (a ones column in v) is the softmax
    denominator
  - normalize by the reciprocal denominator, then output projection.

The alibi bias of this module is min(a_h*(s-t), 0) (tril overwrites the
causal mask in the torch reference, so future tokens are attended with bias
0), hence P = exp(score/8) * min(exp(a*(s-t)), 1), which factors into a
per-partition ACT bias exp(a*(s_in-127)) and a distance-only (Toeplitz) row
multiplier exp(-a*delta) applied per region (past/diag/future).
"""

import math
import numpy as np

# ---- problem constants (hardcoded; kernel.py must be self-contained) ----
B, T, C = 4, 2048, 1024
N_HEAD, N_KV_HEAD, HEAD_DIM = 16, 4, 64
NH = 8            # heads per core
ST = T // 128     # 16 s-tiles
NCH = T // 512    # 4 t-chunks
KCT = C // 128    # 8 contraction tiles of 128
WREP_W = 2048     # Toeplitz table width: index = t - 128*j is always < 2048
CUT_MARGIN = 10.0  # exp(-10) ~ 4.5e-5: dropped mass is ~1e-4 of denom

_START = 2.0 ** (-2.0 ** (-(math.log2(N_HEAD) - 3.0)))  # 0.7071...


def _head_of_slot(i: int, g: int) -> int:
    return 4 * (i // 2) + 2 * g + (i % 2)


def _a_of_head(h: int) -> float:
    return (_START ** (h + 1)) / math.sqrt(HEAD_DIM)


# Loop bounds must be identical on every core (SPMD): use the widest cutoff
# over g for each head slot (g=1 heads have smaller slopes -> wider bands).
_CUTOFF = [CUT_MARGIN / min(_a_of_head(_head_of_slot(i, 0)),
                            _a_of_head(_head_of_slot(i, 1)))
           for i in range(NH)]
_N_EFF = [[min(NCH, int((128 * j + _CUTOFF[i]) // 512) + 1)
           for j in range(ST)] for i in range(NH)]
_J_FIRST = [[min(j for j in range(ST) if _N_EFF[i][j] > tcn)
             for tcn in range(NCH)] for i in range(NH)]

_NC_CACHE = {}


def _split_multiwait(nc, mybir, max_waits=1):
    """walrus in this env encodes at most one sync-wait per instruction;
    split extras onto same-engine NoOps emitted just before."""
    for f in nc.m.functions:
        for bb in f.blocks:
            new = []
            for ins in bb.instructions:
                si = ins.sync_info
                conds = list(si.on_wait) if si is not None else []
                if len(conds) > max_waits:
                    for cond in conds[:-max_waits]:
                        n = mybir.InstNoOp(
                            name=nc.get_next_instruction_name(), ins=[], outs=[])
                        n.engine = ins.engine
                        n.sync_info = mybir.SyncInfo(on_wait=[cond], on_update=[])
                        new.append(n)
                    si.on_wait = conds[-max_waits:]
                new.append(ins)
            bb.instructions = new


def _build_nc():
    if "nc" in _NC_CACHE:
        return _NC_CACHE["nc"]
    import concourse.bass as bass
    import concourse.tile as tile
    from concourse import mybir

    f32 = mybir.dt.float32
    bf16 = mybir.dt.bfloat16
    AF = mybir.ActivationFunctionType
    MUL = mybir.AluOpType.mult
    MIN = mybir.AluOpType.min

    nc = bass.Bass()

    xT_d = nc.dram_tensor("xT", [C, T], bf16, kind="ExternalInput")
    wq_d = nc.dram_tensor("wqT", [C, NH * 64], bf16, kind="ExternalInput")
    wk_d = nc.dram_tensor("wkT", [C, 128], bf16, kind="ExternalInput")
    wv_d = nc.dram_tensor("wvT", [C, 128], bf16, kind="ExternalInput")
    wp_d = nc.dram_tensor("wpT", [NH * 64, C], bf16, kind="ExternalInput")
    wrep_d = nc.dram_tensor("wrep", [NH, 128, WREP_W], bf16, kind="ExternalInput")
    u_d = nc.dram_tensor("usb", [128, NH], f32, kind="ExternalInput")
    bias_d = nc.dram_tensor("biassb", [128, NH], f32, kind="ExternalInput")
    out_d = nc.dram_tensor("out", [T, C], f32, kind="ExternalOutput")

    xT_r = xT_d.rearrange("(k p) t -> p k t", p=128)
    wq_r = wq_d.rearrange("(k p) e -> p k e", p=128)

    with tile.TileContext(nc) as tc:
        with (
            tc.tile_pool(name="const", bufs=1) as const,
        ):
            # ---- constant tiles (loads emitted in dependency-critical order) ----
            wp = const.tile([128, 4, C], bf16)
            wrep = const.tile([128, NH, WREP_W], bf16)
            wrep_r = wrep_d.rearrange("h p w -> p h w")
            usb = const.tile([128, NH], f32)
            biassb = const.tile([128, NH], f32)

            kRep = const.tile([128, 2, T], bf16)     # kv on both halves
            v_sb = const.tile([128, ST, 130], bf16)  # [s, j, (v_kv0|1|v_kv1|1)]
            qRep = const.tile([128, NH, T], bf16)    # head i on both halves
            outT = const.tile([128, 4, T], bf16)     # [(2 heads d), pair, t]
            dstack = const.tile([128, 128], bf16)    # [(head,tt), t_in] denom

            # ---- phase 1: projections (inputs in a transient pool) ----
            with (
                tc.tile_pool(name="ph1", bufs=1) as ph1,
                tc.tile_pool(name="psq", bufs=2, space="PSUM") as psq,
                tc.tile_pool(name="psv", bufs=2, space="PSUM") as psv,
            ):
                xT = ph1.tile([128, KCT, T], bf16)
                wk = ph1.tile([128, KCT, 128], bf16)
                nc.sync.dma_start(out=wk, in_=wk_d.rearrange("(k p) e -> p k e", p=128))
                wq = ph1.tile([128, KCT, NH * 64], bf16)
                for tb in range(4):
                    for kc in range(KCT):
                        nc.sync.dma_start(out=xT[:, kc, 512 * tb:512 * (tb + 1)],
                                          in_=xT_r[:, kc, 512 * tb:512 * (tb + 1)])
                    if tb == 0:
                        nc.gpsimd.dma_start(out=wq[:, :, 0:256],
                                            in_=wq_r[:, :, 0:256])
                    elif tb == 1:
                        nc.gpsimd.dma_start(out=wq[:, :, 256:512],
                                            in_=wq_r[:, :, 256:512])
                nc.gpsimd.dma_start(out=usb, in_=u_d[:])
                nc.gpsimd.dma_start(out=biassb, in_=bias_d[:])
                nc.gpsimd.dma_start(out=wrep[:, 0, :], in_=wrep_r[:, 0, :])
                wv = ph1.tile([128, KCT, 128], bf16)
                nc.gpsimd.dma_start(out=wv, in_=wv_d.rearrange("(k p) e -> p k e", p=128))
                for i in range(1, NH):
                    nc.gpsimd.dma_start(out=wrep[:, i, :], in_=wrep_r[:, i, :])
                nc.gpsimd.dma_start(out=wp, in_=wp_d.rearrange("(k p) e -> p k e", p=128))

                # k projection -> kRep (kv0 on low half slot0, kv1 high slot1)
                def k_proj_chunk(sc):
                    ps = psq.tile([128, 512], f32, tag="pk")
                    for kc in range(KCT):
                        nc.tensor.matmul(
                            ps, lhsT=wk[:, kc, :],
                            rhs=xT[:, kc, 512 * sc:512 * (sc + 1)],
                            start=(kc == 0), stop=(kc == KCT - 1))
                    sl = slice(512 * sc, 512 * (sc + 1))
                    nc.vector.tensor_copy(kRep[0:64, 0, sl], ps[0:64, :])
                    nc.vector.tensor_copy(kRep[64:128, 1, sl], ps[64:128, :])

                def q_proj_pair(p):
                    for tcn in range(NCH):
                        ps = psq.tile([128, 512], f32, tag="pk")
                        for kc in range(KCT):
                            nc.tensor.matmul(
                                ps, lhsT=wq[:, kc, 128 * p:128 * (p + 1)],
                                rhs=xT[:, kc, 512 * tcn:512 * (tcn + 1)],
                                start=(kc == 0), stop=(kc == KCT - 1))
                        sl = slice(512 * tcn, 512 * (tcn + 1))
                        nc.vector.tensor_copy(qRep[0:64, 2 * p, sl], ps[0:64, :])
                        nc.vector.tensor_copy(qRep[64:128, 2 * p + 1, sl],
                                              ps[64:128, :])
                    nc.sync.dma_start(out=qRep[64:128, 2 * p, :],
                                      in_=qRep[0:64, 2 * p, :])
                    nc.sync.dma_start(out=qRep[0:64, 2 * p + 1, :],
                                      in_=qRep[64:128, 2 * p + 1, :])

                for sc in range(NCH):
                    k_proj_chunk(sc)
                nc.sync.dma_start(out=kRep[64:128, 0, :], in_=kRep[0:64, 0, :])
                nc.sync.dma_start(out=kRep[0:64, 1, :], in_=kRep[64:128, 1, :])
                q_proj_pair(0)
                # v projection -> v_sb with ones columns
                for st in range(ST):
                    ps = psv.tile([128, 128], f32, tag="pv")
                    for kc in range(KCT):
                        nc.tensor.matmul(
                            ps, lhsT=xT[:, kc, 128 * st:128 * (st + 1)],
                            rhs=wv[:, kc, :],
                            start=(kc == 0), stop=(kc == KCT - 1))
                    nc.vector.tensor_copy(v_sb[:, st, 0:64], ps[:, 0:64])
                    nc.vector.tensor_copy(v_sb[:, st, 65:129], ps[:, 64:128])
                nc.vector.memset(v_sb[:, :, 64], 1.0)
                nc.vector.memset(v_sb[:, :, 129], 1.0)
                for p in range(1, 4):
                    q_proj_pair(p)

            # ---- phase 2: attention per head ----
            with (
                tc.tile_pool(name="work", bufs=3) as work,
                tc.tile_pool(name="ebuf", bufs=3) as ebufp,
                tc.tile_pool(name="outp", bufs=2) as outp,
                tc.tile_pool(name="dstgp", bufs=2) as dstgp,
                tc.tile_pool(name="dramd", bufs=2, space="DRAM") as dramd,
            ):
              ddrow = dramd.tile([NH, T], bf16)
              # psA opened first -> banks 0-3 (overlap phase-1 psq/psv banks,
              # attnv starts late anyway); psS -> banks 4-7 (no phase-1 WAR,
              # so head-0 scores overlap the tail of the projections).
              with (
                tc.tile_pool(name="psA", bufs=1, space="PSUM") as psA,
                tc.tile_pool(name="psS", bufs=2, space="PSUM") as psS,
              ):
                for i in range(NH):
                    p, half = i // 2, i % 2
                    pa = psA.tile([65, T], f32, tag="pa")
                    # diag multiplier min(exp(-a(t_in-127)), exp(a(127-s_in)))
                    dmin = work.tile([128, 128], bf16, tag="dmin")
                    nc.vector.tensor_scalar(dmin, wrep[:, i, 0:128],
                                            usb[:, i:i + 1], None, MIN)
                    for j in range(ST):
                        ne = _N_EFF[i][j]
                        W = 512 * ne
                        E = ebufp.tile([128, T], bf16, tag="E")
                        lo = 128 * j         # t < lo : future (bias 0)
                        hi = 128 * (j + 1)   # t >= hi: past (Toeplitz)
                        for sh in range(2):
                            c0, c1 = 2 * sh, min(ne, 2 * sh + 2)
                            if c0 >= c1:
                                continue
                            base, top = 512 * c0, 512 * c1
                            S = psS.tile([128, 1024], f32, tag="S")
                            for tcn in range(c0, c1):
                                rh = 64 * (tcn % 2)
                                o = 512 * (tcn - c0)
                                nc.tensor.matmul(
                                    S[:, o:o + 512],
                                    lhsT=kRep[rh:rh + 64, half,
                                              128 * j:128 * (j + 1)],
                                    rhs=qRep[rh:rh + 64, i,
                                             512 * tcn:512 * (tcn + 1)],
                                    start=True, stop=True)
                            wv_ = top - base
                            if lo >= top:
                                # fully-future tile: alibi bias is 0 there
                                nc.scalar.activation(
                                    E[:, base:top], S[:, :wv_], AF.Exp,
                                    bias=0.0, scale=0.125)
                                continue
                            nc.scalar.activation(
                                E[:, base:top], S[:, :wv_], AF.Exp,
                                bias=biassb[:, i:i + 1], scale=0.125)
                            if lo > base:  # future prefix: cancel the bias
                                nc.vector.tensor_scalar(
                                    E[:, base:lo], E[:, base:lo],
                                    usb[:, i:i + 1], None, MUL)
                            if lo >= base:  # diagonal tile lives here
                                nc.vector.tensor_tensor(
                                    E[:, lo:hi], E[:, lo:hi], dmin, MUL)
                            seg0 = max(hi, base)
                            if seg0 < top:  # past region: Toeplitz mult
                                nc.vector.tensor_tensor(
                                    E[:, seg0:top], E[:, seg0:top],
                                    wrep[:, i, 128 + seg0 - hi:
                                         128 + top - hi], MUL)
                        for tcn in range(ne):
                            nc.tensor.matmul(
                                pa[:, 512 * tcn:512 * (tcn + 1)],
                                lhsT=v_sb[:, j, 65 * half:65 * half + 65],
                                rhs=E[:, 512 * tcn:512 * (tcn + 1)],
                                start=(j == _J_FIRST[i][tcn]), stop=(j == ST - 1),
                                skip_group_check=True)
                    # copy-out: rows 0:64 -> outT half; row 64 -> denom
                    st65 = dstgp.tile([65, T], bf16, tag="st65")
                    nc.vector.tensor_copy(st65, pa[0:65, :])
                    nc.sync.dma_start(out=outT[64 * half:64 * half + 64, p, :],
                                      in_=st65[0:64, :])
                    nc.sync.dma_start(out=ddrow[i:i + 1, :], in_=st65[64:65, :])
                    nc.sync.dma_start(
                        out=dstack[16 * i:16 * (i + 1), :],
                        in_=ddrow[i].rearrange("(a b) -> a b", b=128))

              # ---- phase 3: normalize + output projection ----
              dstf = const.tile([128, 128], f32)
              nc.vector.tensor_copy(dstf, dstack)
              rstf = const.tile([128, 128], f32)
              nc.vector.reciprocal(rstf, dstf)
              rstb = const.tile([128, 128], bf16)
              nc.vector.tensor_copy(rstb, rstf)
              with (
                  tc.tile_pool(name="dramr", bufs=1, space="DRAM") as dramr,
                  tc.tile_pool(name="psP", bufs=2, space="PSUM") as psP,
              ):
                rdram = dramr.tile([NH, T], bf16)
                rd3 = rdram.rearrange("i (a b) -> i a b", b=128)
                for i in range(NH):
                    nc.sync.dma_start(out=rd3[i], in_=rstb[16 * i:16 * (i + 1), :])
                for p in range(4):
                    rrep = work.tile([128, T], bf16, tag="rrep")
                    for hh in range(2):
                        i = 2 * p + hh
                        src = rdram[i:i + 1, :]
                        src = bass.AP(tensor=src.tensor, offset=src.offset,
                                      ap=[[0, 64]] + list(src.ap)[1:])
                        nc.sync.dma_start(out=rrep[64 * hh:64 * hh + 64, :],
                                          in_=src)
                    nc.vector.tensor_tensor(outT[:, p, :], outT[:, p, :], rrep,
                                            MUL)
                for tt in range(ST):
                    osb = outp.tile([128, C], f32, tag="osb")
                    for ec in range(2):
                        ps = psP.tile([128, 512], f32, tag="pp")
                        for kt in range(4):
                            nc.tensor.matmul(
                                ps, lhsT=outT[:, kt, 128 * tt:128 * (tt + 1)],
                                rhs=wp[:, kt, 512 * ec:512 * (ec + 1)],
                                start=(kt == 0), stop=(kt == 3))
                        nc.vector.tensor_copy(osb[:, 512 * ec:512 * (ec + 1)], ps)
                    eng = nc.sync if tt % 2 == 0 else nc.gpsimd
                    eng.dma_start(out=out_d[128 * tt:128 * (tt + 1), :],
                                  in_=osb)

    _split_multiwait(nc, mybir)
    _NC_CACHE["nc"] = nc
    return nc


def _prep_core_inputs(x, Wq, Wkv, Wproj, b, g):
    import ml_dtypes
    bf = ml_dtypes.bfloat16
    heads = [_head_of_slot(i, g) for i in range(NH)]
    xT = np.ascontiguousarray(x[b].T).astype(bf)                      # [C, T]
    wq_cols = np.concatenate([Wq[64 * h:64 * (h + 1)] for h in heads], axis=0)
    wqT = np.ascontiguousarray(wq_cols.T).astype(bf)                  # [C, 512]
    wkT = np.ascontiguousarray(Wkv[128 * g:128 * (g + 1)].T).astype(bf)
    wvT = np.ascontiguousarray(Wkv[256 + 128 * g:256 + 128 * (g + 1)].T).astype(bf)
    cols = np.concatenate([np.arange(64 * h, 64 * (h + 1)) for h in heads])
    wpT = np.ascontiguousarray(Wproj[:, cols].T).astype(bf)           # [512, C]

    s_in = np.arange(128, dtype=np.float64)
    wrep = np.empty((NH, 128, WREP_W), dtype=bf)
    u = np.empty((128, NH), dtype=np.float32)
    bias = np.empty((128, NH), dtype=np.float32)
    idx = np.arange(WREP_W, dtype=np.float64)
    for i, h in enumerate(heads):
        a = _a_of_head(h)
        wrep[i] = np.exp(-a * (idx - 127.0))[None, :].astype(np.float32)
        u[:, i] = np.exp(a * (127.0 - s_in)).astype(np.float32)
        bias[:, i] = (a * (s_in - 127.0)).astype(np.float32)
    return {"xT": xT, "wqT": wqT, "wkT": wkT, "wvT": wvT, "wpT": wpT,
            "wrep": wrep, "usb": u, "biassb": bias}


def kernel(x, Wq, Wkv, Wproj, bproj):
    from concourse.bass_utils import run_bass_kernel_spmd
    x = np.asarray(x, dtype=np.float32)
    Wq = np.asarray(Wq, dtype=np.float32)
    Wkv = np.asarray(Wkv, dtype=np.float32)
    Wproj = np.asarray(Wproj, dtype=np.float32)
    bproj = np.asarray(bproj, dtype=np.float32)

    nc = _build_nc()
    in_maps = [_prep_core_inputs(x, Wq, Wkv, Wproj, c // 2, c % 2)
               for c in range(8)]
    res = run_bass_kernel_spmd(nc, in_maps, core_ids=list(range(8)))
    out = np.zeros((B, T, C), dtype=np.float32)
    for c in range(8):
        out[c // 2] += res.results[c]["out"]
    out += bproj[None, None, :]
    return out



# revision 20
# speedup vs baseline: 1.3170x; 1.0027x over previous
"""Bass/Tile TRN2 kernel for nn_DecoderGroupedQueryHeadAttentionAlibi.

Sharding (8 cores): core = (b, g) with b = core//2 in [0,4) (batch),
g = core%2 (head-half). Each core computes 8 of 16 query heads (those with
h%4 in {2g, 2g+1}) for its batch, plus the corresponding row-slice of the
output projection; the host sums the two half partials and adds bproj.

Per-core device program (layout A, scoresT = [s_partitions, t_free]):
  - projections q/k/v from host-pretransposed xT/weight tiles (bf16 matmuls)
  - per (head, s-tile): scoresT psum [128,2048] -> ACT exp (alibi linear bias
    folded into the activation per-partition bias) -> DVE multiplier using a
    Toeplitz exp table sliced at a per-s-tile offset -> attn@v accumulation
    in psum [65,2048] where row 64 (a ones column in v) is the softmax
    denominator
  - normalize by the reciprocal denominator, then output projection.

The alibi bias of this module is min(a_h*(s-t), 0) (tril overwrites the
causal mask in the torch reference, so future tokens are attended with bias
0), hence P = exp(score/8) * min(exp(a*(s-t)), 1), which factors into a
per-partition ACT bias exp(a*(s_in-127)) and a distance-only (Toeplitz) row
multiplier exp(-a*delta) applied per region (past/diag/future).
"""

import math
import numpy as np

# ---- problem constants (hardcoded; kernel.py must be self-contained) ----
B, T, C = 4, 2048, 1024
N_HEAD, N_KV_HEAD, HEAD_DIM = 16, 4, 64
NH = 8            # heads per core
ST = T // 128     # 16 s-tiles
NCH = T // 512    # 4 t-chunks
KCT = C // 128    # 8 contraction tiles of 128
WREP_W = 2048     # Toeplitz table width: index = t - 128*j is always < 2048
CUT_MARGIN = 10.0  # exp(-10) ~ 4.5e-5: dropped mass is ~1e-4 of denom

_START = 2.0 ** (-2.0 ** (-(math.log2(N_HEAD) - 3.0)))  # 0.7071...


def _head_of_slot(i: int, g: int) -> int:
    return 4 * (i // 2) + 2 * g + (i % 2)


def _a_of_head(h: int) -> float:
    return (_START ** (h + 1)) / math.sqrt(HEAD_DIM)


# Loop bounds must be identical on every core (SPMD): use the widest cutoff
# over g for each head slot (g=1 heads have smaller slopes -> wider bands).
_CUTOFF = [CUT_MARGIN / min(_a_of_head(_head_of_slot(i, 0)),
                            _a_of_head(_head_of_slot(i, 1)))
           for i in range(NH)]
_N_EFF = [[min(NCH, int((128 * j + _CUTOFF[i]) // 512) + 1)
           for j in range(ST)] for i in range(NH)]
_J_FIRST = [[min(j for j in range(ST) if _N_EFF[i][j] > tcn)
             for tcn in range(NCH)] for i in range(NH)]

_NC_CACHE = {}


def _split_multiwait(nc, mybir, max_waits=1):
    """walrus in this env encodes at most one sync-wait per instruction;
    split extras onto same-engine NoOps emitted just before."""
    for f in nc.m.functions:
        for bb in f.blocks:
            new = []
            for ins in bb.instructions:
                si = ins.sync_info
                conds = list(si.on_wait) if si is not None else []
                if len(conds) > max_waits:
                    for cond in conds[:-max_waits]:
                        n = mybir.InstNoOp(
                            name=nc.get_next_instruction_name(), ins=[], outs=[])
                        n.engine = ins.engine
                        n.sync_info = mybir.SyncInfo(on_wait=[cond], on_update=[])
                        new.append(n)
                    si.on_wait = conds[-max_waits:]
                new.append(ins)
            bb.instructions = new


def _build_nc():
    if "nc" in _NC_CACHE:
        return _NC_CACHE["nc"]
    import concourse.bass as bass
    import concourse.tile as tile
    from concourse import mybir

    f32 = mybir.dt.float32
    bf16 = mybir.dt.bfloat16
    AF = mybir.ActivationFunctionType
    MUL = mybir.AluOpType.mult
    MIN = mybir.AluOpType.min

    nc = bass.Bass()

    xT_d = nc.dram_tensor("xT", [C, T], bf16, kind="ExternalInput")
    wq_d = nc.dram_tensor("wqT", [C, NH * 64], bf16, kind="ExternalInput")
    wk_d = nc.dram_tensor("wkT", [C, 128], bf16, kind="ExternalInput")
    wv_d = nc.dram_tensor("wvT", [C, 128], bf16, kind="ExternalInput")
    wp_d = nc.dram_tensor("wpT", [NH * 64, C], bf16, kind="ExternalInput")
    wrep_d = nc.dram_tensor("wrep", [NH, 128, WREP_W], bf16, kind="ExternalInput")
    u_d = nc.dram_tensor("usb", [128, NH], f32, kind="ExternalInput")
    bias_d = nc.dram_tensor("biassb", [128, NH], f32, kind="ExternalInput")
    out_d = nc.dram_tensor("out", [T, C], f32, kind="ExternalOutput")

    xT_r = xT_d.rearrange("(k p) t -> p k t", p=128)
    wq_r = wq_d.rearrange("(k p) e -> p k e", p=128)

    with tile.TileContext(nc) as tc:
        with (
            tc.tile_pool(name="const", bufs=1) as const,
        ):
            # ---- constant tiles (loads emitted in dependency-critical order) ----
            wp = const.tile([128, 4, C], bf16)
            wrep = const.tile([128, NH, WREP_W], bf16)
            wrep_r = wrep_d.rearrange("h p w -> p h w")
            usb = const.tile([128, NH], f32)
            biassb = const.tile([128, NH], f32)

            kRep = const.tile([128, 2, T], bf16)     # kv on both halves
            v_sb = const.tile([128, ST, 130], bf16)  # [s, j, (v_kv0|1|v_kv1|1)]
            qRep = const.tile([128, NH, T], bf16)    # head i on both halves
            outT = const.tile([128, 4, T], bf16)     # [(2 heads d), pair, t]
            dstack = const.tile([128, 128], bf16)    # [(head,tt), t_in] denom

            # ---- phase 1: projections (inputs in a transient pool) ----
            with (
                tc.tile_pool(name="ph1", bufs=1) as ph1,
                tc.tile_pool(name="psq", bufs=2, space="PSUM") as psq,
                tc.tile_pool(name="psv", bufs=2, space="PSUM") as psv,
            ):
                xT = ph1.tile([128, KCT, T], bf16)
                wk = ph1.tile([128, KCT, 128], bf16)
                nc.sync.dma_start(out=wk, in_=wk_d.rearrange("(k p) e -> p k e", p=128))
                wq = ph1.tile([128, KCT, NH * 64], bf16)
                for tb in range(4):
                    for kc in range(KCT):
                        nc.sync.dma_start(out=xT[:, kc, 512 * tb:512 * (tb + 1)],
                                          in_=xT_r[:, kc, 512 * tb:512 * (tb + 1)])
                    if tb == 0:
                        nc.gpsimd.dma_start(out=wq[:, :, 0:256],
                                            in_=wq_r[:, :, 0:256])
                    elif tb == 1:
                        nc.gpsimd.dma_start(out=wq[:, :, 256:512],
                                            in_=wq_r[:, :, 256:512])
                nc.gpsimd.dma_start(out=usb, in_=u_d[:])
                nc.gpsimd.dma_start(out=biassb, in_=bias_d[:])
                nc.gpsimd.dma_start(out=wrep[:, 0, :], in_=wrep_r[:, 0, :])
                wv = ph1.tile([128, KCT, 128], bf16)
                nc.gpsimd.dma_start(out=wv, in_=wv_d.rearrange("(k p) e -> p k e", p=128))
                for i in range(1, NH):
                    nc.gpsimd.dma_start(out=wrep[:, i, :], in_=wrep_r[:, i, :])
                nc.gpsimd.dma_start(out=wp, in_=wp_d.rearrange("(k p) e -> p k e", p=128))

                # k projection -> kRep (kv0 on low half slot0, kv1 high slot1)
                def k_proj_chunk(sc):
                    ps = psq.tile([128, 512], f32, tag="pk")
                    for kc in range(KCT):
                        nc.tensor.matmul(
                            ps, lhsT=wk[:, kc, :],
                            rhs=xT[:, kc, 512 * sc:512 * (sc + 1)],
                            start=(kc == 0), stop=(kc == KCT - 1))
                    sl = slice(512 * sc, 512 * (sc + 1))
                    nc.vector.tensor_copy(kRep[0:64, 0, sl], ps[0:64, :])
                    nc.vector.tensor_copy(kRep[64:128, 1, sl], ps[64:128, :])

                def q_proj_pair(p):
                    for tcn in range(NCH):
                        ps = psq.tile([128, 512], f32, tag="pk")
                        for kc in range(KCT):
                            nc.tensor.matmul(
                                ps, lhsT=wq[:, kc, 128 * p:128 * (p + 1)],
                                rhs=xT[:, kc, 512 * tcn:512 * (tcn + 1)],
                                start=(kc == 0), stop=(kc == KCT - 1))
                        sl = slice(512 * tcn, 512 * (tcn + 1))
                        nc.vector.tensor_copy(qRep[0:64, 2 * p, sl], ps[0:64, :])
                        nc.vector.tensor_copy(qRep[64:128, 2 * p + 1, sl],
                                              ps[64:128, :])
                    nc.sync.dma_start(out=qRep[64:128, 2 * p, :],
                                      in_=qRep[0:64, 2 * p, :])
                    nc.sync.dma_start(out=qRep[0:64, 2 * p + 1, :],
                                      in_=qRep[64:128, 2 * p + 1, :])

                for sc in range(NCH):
                    k_proj_chunk(sc)
                nc.sync.dma_start(out=kRep[64:128, 0, :], in_=kRep[0:64, 0, :])
                nc.sync.dma_start(out=kRep[0:64, 1, :], in_=kRep[64:128, 1, :])
                q_proj_pair(0)
                # v projection -> v_sb with ones columns
                for st in range(ST):
                    ps = psv.tile([128, 128], f32, tag="pv")
                    for kc in range(KCT):
                        nc.tensor.matmul(
                            ps, lhsT=xT[:, kc, 128 * st:128 * (st + 1)],
                            rhs=wv[:, kc, :],
                            start=(kc == 0), stop=(kc == KCT - 1))
                    nc.vector.tensor_copy(v_sb[:, st, 0:64], ps[:, 0:64])
                    nc.vector.tensor_copy(v_sb[:, st, 65:129], ps[:, 64:128])
                nc.vector.memset(v_sb[:, :, 64], 1.0)
                nc.vector.memset(v_sb[:, :, 129], 1.0)
                for p in range(1, 4):
                    q_proj_pair(p)

            # ---- phase 2: attention per head ----
            with (
                tc.tile_pool(name="work", bufs=3) as work,
                tc.tile_pool(name="ebuf", bufs=3) as ebufp,
                tc.tile_pool(name="outp", bufs=2) as outp,
                tc.tile_pool(name="dstgp", bufs=2) as dstgp,
                tc.tile_pool(name="dramd", bufs=2, space="DRAM") as dramd,
            ):
              ddrow = dramd.tile([NH, T], bf16)
              rdram = dramd.tile([NH, T], bf16, bufs=1)
              rd3 = rdram.rearrange("i (a b) -> i a b", b=128)
              dstf = const.tile([128, 128], f32)
              rstf = const.tile([128, 128], f32)
              rstb = const.tile([128, 128], bf16)

              def finish_pair(p_):
                  # denoms of heads 2p_,2p_+1 are in dstack: recip + normalize
                  sl = slice(32 * p_, 32 * p_ + 32)
                  nc.vector.tensor_copy(dstf[sl, :], dstack[sl, :])
                  nc.vector.reciprocal(rstf[sl, :], dstf[sl, :])
                  nc.vector.tensor_copy(rstb[sl, :], rstf[sl, :])
                  for hh in range(2):
                      i_ = 2 * p_ + hh
                      nc.gpsimd.dma_start(out=rd3[i_],
                                          in_=rstb[16 * i_:16 * i_ + 16, :])
                  rrep = work.tile([128, T], bf16, tag="rrep")
                  for hh in range(2):
                      i_ = 2 * p_ + hh
                      src = rdram[i_:i_ + 1, :]
                      src = bass.AP(tensor=src.tensor, offset=src.offset,
                                    ap=[[0, 64]] + list(src.ap)[1:])
                      nc.gpsimd.dma_start(out=rrep[64 * hh:64 * hh + 64, :],
                                          in_=src)
                  nc.vector.tensor_tensor(outT[:, p_, :], outT[:, p_, :],
                                          rrep, MUL)

              # psA opened first -> banks 0-3 (overlap phase-1 psq/psv banks,
              # attnv starts late anyway); psS -> banks 4-7 (no phase-1 WAR,
              # so head-0 scores overlap the tail of the projections).
              with (
                tc.tile_pool(name="psA", bufs=1, space="PSUM") as psA,
                tc.tile_pool(name="psS", bufs=2, space="PSUM") as psS,
              ):
                for i in range(NH):
                    p, half = i // 2, i % 2
                    pa = psA.tile([65, T], f32, tag="pa")
                    # diag multiplier min(exp(-a(t_in-127)), exp(a(127-s_in)))
                    dmin = work.tile([128, 128], bf16, tag="dmin")
                    nc.vector.tensor_scalar(dmin, wrep[:, i, 0:128],
                                            usb[:, i:i + 1], None, MIN)
                    for j in range(ST):
                        ne = _N_EFF[i][j]
                        W = 512 * ne
                        E = ebufp.tile([128, T], bf16, tag="E")
                        lo = 128 * j         # t < lo : future (bias 0)
                        hi = 128 * (j + 1)   # t >= hi: past (Toeplitz)
                        for sh in range(2):
                            c0, c1 = 2 * sh, min(ne, 2 * sh + 2)
                            if c0 >= c1:
                                continue
                            base, top = 512 * c0, 512 * c1
                            S = psS.tile([128, 1024], f32, tag="S")
                            for tcn in range(c0, c1):
                                rh = 64 * (tcn % 2)
                                o = 512 * (tcn - c0)
                                nc.tensor.matmul(
                                    S[:, o:o + 512],
                                    lhsT=kRep[rh:rh + 64, half,
                                              128 * j:128 * (j + 1)],
                                    rhs=qRep[rh:rh + 64, i,
                                             512 * tcn:512 * (tcn + 1)],
                                    start=True, stop=True)
                            wv_ = top - base
                            if lo >= top:
                                # fully-future tile: alibi bias is 0 there
                                nc.scalar.activation(
                                    E[:, base:top], S[:, :wv_], AF.Exp,
                                    bias=0.0, scale=0.125)
                                continue
                            nc.scalar.activation(
                                E[:, base:top], S[:, :wv_], AF.Exp,
                                bias=biassb[:, i:i + 1], scale=0.125)
                            if lo > base:  # future prefix: cancel the bias
                                nc.vector.tensor_scalar(
                                    E[:, base:lo], E[:, base:lo],
                                    usb[:, i:i + 1], None, MUL)
                            if lo >= base:  # diagonal tile lives here
                                nc.vector.tensor_tensor(
                                    E[:, lo:hi], E[:, lo:hi], dmin, MUL)
                            seg0 = max(hi, base)
                            if seg0 < top:  # past region: Toeplitz mult
                                nc.vector.tensor_tensor(
                                    E[:, seg0:top], E[:, seg0:top],
                                    wrep[:, i, 128 + seg0 - hi:
                                         128 + top - hi], MUL)
                        for tcn in range(ne):
                            nc.tensor.matmul(
                                pa[:, 512 * tcn:512 * (tcn + 1)],
                                lhsT=v_sb[:, j, 65 * half:65 * half + 65],
                                rhs=E[:, 512 * tcn:512 * (tcn + 1)],
                                start=(j == _J_FIRST[i][tcn]), stop=(j == ST - 1),
                                skip_group_check=True)
                    # copy-out: rows 0:64 -> outT half; row 64 -> denom
                    st65 = dstgp.tile([65, T], bf16, tag="st65")
                    nc.vector.tensor_copy(st65, pa[0:65, :])
                    nc.sync.dma_start(out=outT[64 * half:64 * half + 64, p, :],
                                      in_=st65[0:64, :])
                    nc.sync.dma_start(out=ddrow[i:i + 1, :], in_=st65[64:65, :])
                    nc.sync.dma_start(
                        out=dstack[16 * i:16 * (i + 1), :],
                        in_=ddrow[i].rearrange("(a b) -> a b", b=128))
                    if half == 1:
                        finish_pair(p)

              # ---- phase 3: output projection (normalize already done) ----
              with (
                  tc.tile_pool(name="psP", bufs=4, space="PSUM") as psP,
              ):
                for tt in range(ST):
                    osb = outp.tile([128, C], f32, tag="osb")
                    for ec in range(2):
                        pp = psP.tile([128, 512], f32, tag="pp")
                        for kt in range(4):
                            nc.tensor.matmul(
                                pp, lhsT=outT[:, kt, 128 * tt:128 * (tt + 1)],
                                rhs=wp[:, kt, 512 * ec:512 * (ec + 1)],
                                start=(kt == 0), stop=(kt == 3))
                        nc.vector.tensor_copy(osb[:, 512 * ec:512 * (ec + 1)],
                                              pp)
                    eng = nc.sync if tt % 2 == 0 else nc.gpsimd
                    eng.dma_start(out=out_d[128 * tt:128 * (tt + 1), :],
                                  in_=osb)

    _split_multiwait(nc, mybir)
    _NC_CACHE["nc"] = nc
    return nc


def _prep_core_inputs(x, Wq, Wkv, Wproj, b, g):
    import ml_dtypes
    bf = ml_dtypes.bfloat16
    heads = [_head_of_slot(i, g) for i in range(NH)]
    xT = np.ascontiguousarray(x[b].T).astype(bf)                      # [C, T]
    wq_cols = np.concatenate([Wq[64 * h:64 * (h + 1)] for h in heads], axis=0)
    wqT = np.ascontiguousarray(wq_cols.T).astype(bf)                  # [C, 512]
    wkT = np.ascontiguousarray(Wkv[128 * g:128 * (g + 1)].T).astype(bf)
    wvT = np.ascontiguousarray(Wkv[256 + 128 * g:256 + 128 * (g + 1)].T).astype(bf)
    cols = np.concatenate([np.arange(64 * h, 64 * (h + 1)) for h in heads])
    wpT = np.ascontiguousarray(Wproj[:, cols].T).astype(bf)           # [512, C]

    s_in = np.arange(128, dtype=np.float64)
    wrep = np.empty((NH, 128, WREP_W), dtype=bf)
    u = np.empty((128, NH), dtype=np.float32)
    bias = np.empty((128, NH), dtype=np.float32)
    idx = np.arange(WREP_W, dtype=np.float64)
    for i, h in enumerate(heads):
        a = _a_of_head(h)
        wrep[i] = np.exp(-a * (idx - 127.0))[None, :].astype(np.float32)
        u[:, i] = np.exp(a * (127.0 - s_in)).astype(np.float32)
        bias[:, i] = (a * (s_in - 127.0)).astype(np.float32)
    return {"xT": xT, "wqT": wqT, "wkT": wkT, "wvT": wvT, "wpT": wpT,
            "wrep": wrep, "usb": u, "biassb": bias}


def kernel(x, Wq, Wkv, Wproj, bproj):
    from concourse.bass_utils import run_bass_kernel_spmd
    x = np.asarray(x, dtype=np.float32)
    Wq = np.asarray(Wq, dtype=np.float32)
    Wkv = np.asarray(Wkv, dtype=np.float32)
    Wproj = np.asarray(Wproj, dtype=np.float32)
    bproj = np.asarray(bproj, dtype=np.float32)

    nc = _build_nc()
    in_maps = [_prep_core_inputs(x, Wq, Wkv, Wproj, c // 2, c % 2)
               for c in range(8)]
    res = run_bass_kernel_spmd(nc, in_maps, core_ids=list(range(8)))
    out = np.zeros((B, T, C), dtype=np.float32)
    for c in range(8):
        out[c // 2] += res.results[c]["out"]
    out += bproj[None, None, :]
    return out

